# revision 8
# baseline (speedup 1.0000x reference)
"""MoE routed decoder kernel for 8 Trainium2 NeuronCores.

Strategy (v2):
  - Host: compute per-row expert routes, sort rows by expert into EXACT
    (unpadded) segments; NP == B == 1024.
  - Layer 1 is hidden-sharded: each core computes only 2 of the 16 hidden
    128-tiles (its own W1 slice, 1.31MB instead of 10.5MB of HBM reads),
    then all-gathers h SBUF->SBUF over the on-chip fabric with 7
    one-destination remote_dma_broadcast calls (XOR-relative addressing,
    so a single NEFF is valid on every core; the host rotates each core's
    W1/W2 hidden-tile order to match its XOR slot layout).
  - Layer 2 runs TRANSPOSED: W2 128x128 tiles are the stationary operand
    and h columns stream as the moving operand, so PE cost is proportional
    to the real row count (no padding waste): 8 coltiles x 16 ktiles x
    1024 rows = 131K cycles.
  - o^T is copied PSUM->SBUF as bf16 by ACT, transposed back 128x128 by
    the PE, then the complex-pair L2 normalization runs row-major exactly
    as in v1 (ACT square+accum, sqrt, DVE reciprocal + scale).
  - Output is written fp16 (half the HBM write bytes; fp16 mantissa err
    ~5e-4 rel, far inside the 2e-2 budget), host casts back to f32.
  - DMA priority: inputs stream on the SP HWDGE ring in use order
    (xt, ident, W1, then W2 expert by expert); outputs go on the ACT ring
    so they never queue behind W2 loads.
"""

import os
import sys
import types

import numpy as np
import ml_dtypes

import concourse.bass as bass
import concourse.mybir as mybir
import concourse.tile as tile
from concourse import bacc
import concourse.bass_utils as bass_utils
from concourse.bass_utils import run_bass_kernel_spmd
from concourse.tile_rust import add_dep_helper

B, D, H, O, E, P = 1024, 512, 2048, 8192, 5, 128
NCORES = 8
NP = B               # exact row count, no padding
OSL = O // NCORES    # output columns per core
KC1 = D // P         # 4 contraction tiles for layer 1
KC2 = H // P         # 16 contraction tiles for layer 2
NCT = OSL // P       # 8 output column tiles per core
NG = OSL // 256      # 4 norm groups per core slice
NBLK = NP // P       # 8 row blocks
BF16 = mybir.dt.bfloat16
F16 = mybir.dt.float16
F32 = mybir.dt.float32
AF = mybir.ActivationFunctionType

GATHER = os.environ.get("BASSMOE_GATHER", "1") == "1"
NLOC = 2 if GATHER else KC2  # locally computed hidden 128-tiles

LAST_EXEC_NS = None
LAST_TRACE = None


def _install_ntff_hook():
    try:
        import trn_agent_boot.trn_boot as tb

        hook = tb._ntff_profile_via_ctypes("/opt/axon/libaxon_pjrt.so")
        mod = types.ModuleType("antenv.axon_hooks")
        mod.get_axon_ntff_profile_hook = lambda: hook
        import antenv

        antenv.axon_hooks = mod
        sys.modules["antenv.axon_hooks"] = mod
        bass_utils.upload_artifacts = lambda tmpdir: tmpdir
        return True
    except Exception:
        return False


def _route(x):
    c1 = x[:, -1].astype(np.int32) == 0
    c2 = x[:, -2].astype(np.int32) == 0
    c3 = x[:, -3].astype(np.int32) == 0
    r_if = np.where(c2, 0, np.where(c3, 3, 4))
    r_else = np.where(c2, 1, 2)
    return np.where(c1, r_if, r_else).astype(np.int64)


def _plan(route):
    """Exact expert segments in sorted order: [(e, col_start, n_rows)]."""
    pad_idx, segs = [], []
    for e in range(E):
        idx = np.nonzero(route == e)[0]
        if len(idx) == 0:
            continue
        segs.append((e, len(pad_idx), len(idx)))
        pad_idx.extend(idx.tolist())
    return np.array(pad_idx, dtype=np.int64), segs


def _chunks(c0, n, step=512):
    out = []
    off = 0
    while off < n:
        k = min(step, n - off)
        out.append((c0 + off, k))
        off += k
    return out


def _build_program(segs, b1_nz, b2_nz):
    nc = bacc.Bacc("TRN2", target_bir_lowering=False, debug=False,
                   num_devices=NCORES)
    XT = nc.dram_tensor("xt", [P, KC1 * NP], BF16, kind="ExternalInput").ap()
    W1T = nc.dram_tensor("w1", [E, P, KC1 * NLOC * P], BF16,
                         kind="ExternalInput").ap()
    IDT = nc.dram_tensor("idt", [P, P], BF16, kind="ExternalInput").ap()
    W2T = nc.dram_tensor("w2", [E, P, KC2 * OSL], BF16,
                         kind="ExternalInput").ap()
    B1T = nc.dram_tensor("b1", [E, P, NLOC], F32, kind="ExternalInput").ap()
    B2T = nc.dram_tensor("b2", [E, P, NCT], F32, kind="ExternalInput").ap()
    OUT = nc.dram_tensor("out", [NP, OSL], F16, kind="ExternalOutput").ap()

    seg_end = {}
    cum = 0
    for e, c0, n in segs:
        cum = c0 + n
        seg_end[e] = cum
    assert cum == NP

    rsem = lsem = None
    trig_name = None
    l2_names = set()
    if GATHER:
        rsem = nc.alloc_semaphore("hgather")
        lsem = nc.alloc_semaphore("hsend")
        # compile() will insert the kernel-entry barrier AllGather prelude
        # for this replica group; the wait itself is injected post-schedule.
        nc._bir_kernel_barrier_sem_replica_groups.append(set(range(NCORES)))

    with tile.TileContext(nc) as tc:
        with (
            tc.tile_pool(name="singles", bufs=1) as singles,
            tc.tile_pool(name="w1p", bufs=2) as w1p,
            tc.tile_pool(name="w2p", bufs=3) as w2p,
            tc.tile_pool(name="ps1", bufs=2, space="PSUM") as ps1,
            tc.tile_pool(name="ps2", bufs=3, space="PSUM") as ps2,
            tc.tile_pool(name="pst", bufs=2, space="PSUM") as pst,
            tc.tile_pool(name="sqp", bufs=2) as sqp,
            tc.tile_pool(name="outp", bufs=3) as outp,
            tc.tile_pool(name="nrmp", bufs=4) as nrmp,
        ):
            # ---- input DMAs on the SP ring, in priority order ----
            xt_sb = singles.tile([P, KC1, NP], BF16)
            nc.sync.dma_start(xt_sb[:], XT.rearrange("p (k n) -> p k n", k=KC1))
            idt_sb = singles.tile([P, P], BF16)
            nc.sync.dma_start(idt_sb[:], IDT)

            h_sb = singles.tile([P, KC2, NP], BF16)
            oT_sb = singles.tile([P, NCT, NP], BF16)

            b1_sb = None
            if b1_nz:
                b1_sb = singles.tile([P, E, NLOC], F32)
                nc.sync.dma_start(b1_sb[:], B1T.rearrange("e p l -> p e l"))
            b2_sb = None
            if b2_nz:
                b2_sb = singles.tile([P, E, NCT], F32)
                nc.sync.dma_start(b2_sb[:], B2T.rearrange("e p t -> p e t"))

            w1_all = None
            if GATHER:
                w1_all = singles.tile([P, E, KC1, NLOC * P], BF16)
                nc.sync.dma_start(
                    w1_all[:], W1T.rearrange("e p (k q) -> p e k q", k=KC1)
                )

            w2_tiles = {}

            def get_w2(e):
                if e not in w2_tiles:
                    t = w2p.tile([P, KC2, OSL], BF16, tag="w2")
                    nc.sync.dma_start(
                        t[:], W2T[e].rearrange("p (k n) -> p k n", k=KC2)
                    )
                    w2_tiles[e] = t
                return w2_tiles[e]

            # ---- layer 1: local hidden slots over all sorted rows ----
            for ei, (e, c0, n) in enumerate(segs):
                if GATHER:
                    w1t = w1_all[:, e]
                else:
                    w1t = w1p.tile([P, KC1, NLOC * P], BF16, tag="w1")
                    nc.sync.dma_start(
                        w1t[:], W1T[e].rearrange("p (k q) -> p k q", k=KC1)
                    )
                    if ei < 3:
                        get_w2(ei)  # keep W2 streaming behind W1[e]
                for ul in range(NLOC):
                    for cc, nn in _chunks(c0, n):
                        ps = ps1.tile([P, 512], F32, tag="ps1")
                        for kc in range(KC1):
                            nc.tensor.matmul(
                                ps[:, :nn],
                                w1t[:, kc, ul * P:(ul + 1) * P],
                                xt_sb[:, kc, cc:cc + nn],
                                start=(kc == 0),
                                stop=(kc == KC1 - 1),
                            )
                        bias = b1_sb[:, e, ul:ul + 1] if b1_nz else 0.0
                        nc.scalar.activation(
                            h_sb[:, ul, cc:cc + nn], ps[:, :nn], AF.Relu,
                            bias=bias,
                        )

            # ---- all-gather h slots SBUF->SBUF over the fabric ----
            # The barrier wait (before the trigger) and the arrival wait
            # (before the first L2 matmul) are injected into the scheduled
            # stream after the TileContext exits: Tile's single-core
            # scheduling sim cannot satisfy cross-core semaphores.
            if GATHER:
                for dlt in range(1, NCORES):
                    rdests = [None] * 8
                    rdests[dlt] = (0, dlt)
                    # Measured on HW (probe): a send to rdest dlt lands on
                    # core c^dlt for dlt<4 but on c^dlt^2 for dlt>=4 (the
                    # D2D lane pairing adds the ^2). Writing to slot f(dlt)
                    # keeps every receiver's slot s = data of core (self^s).
                    slot = dlt ^ 2 if dlt >= 4 else dlt
                    nc.gpsimd.remote_dma_broadcast(
                        h_sb[:, 2 * slot:2 * slot + 2, :],
                        h_sb[:, 0:2, :],
                        remote_sem=rsem,
                        local_sem=lsem,
                        rdests=rdests,
                    )
                trig = nc.gpsimd.trigger_dma(count=None)
                trig_name = trig.ins.name

            # ---- layer 2 transposed + per-block transpose-back + norm ----
            done_blk = 0

            def emit_blocks(upto):
                nonlocal done_blk
                for r in range(done_blk, upto):
                    pso = pst.tile([P, OSL], BF16, tag="pso")
                    for t in range(NCT):
                        nc.tensor.transpose(
                            pso[:, t * P:(t + 1) * P],
                            oT_sb[:, t, r * P:(r + 1) * P],
                            idt_sb[:],
                        )
                    nrm = nrmp.tile([P, NG], F32, tag="nrm")
                    rn = nrmp.tile([P, NG], F32, tag="rn")
                    out_sb = outp.tile([P, OSL], F16, tag="osb")
                    sq = sqp.tile([P, 256], BF16, tag="sq")
                    for g in range(NG):
                        nc.scalar.activation(
                            sq[:], pso[:, g * 256:(g + 1) * 256], AF.Square,
                            accum_out=nrm[:, g:g + 1],
                        )
                    nc.scalar.sqrt(nrm[:], nrm[:])
                    nc.vector.reciprocal(rn[:], nrm[:])
                    for g in range(NG):
                        nc.vector.tensor_scalar_mul(
                            out_sb[:, g * 256:(g + 1) * 256],
                            pso[:, g * 256:(g + 1) * 256],
                            rn[:, g:g + 1],
                        )
                    nc.scalar.dma_start(OUT[r * P:(r + 1) * P, :], out_sb[:])
                done_blk = upto

            for e, c0, n in segs:
                w2t = get_w2(e)
                for t in range(NCT):
                    for cc, nn in _chunks(c0, n):
                        ps = ps2.tile([P, 512], F32, tag="ps2")
                        for u in range(KC2):
                            mm = nc.tensor.matmul(
                                ps[:, :nn],
                                w2t[:, u, t * P:(t + 1) * P],
                                h_sb[:, u, cc:cc + nn],
                                start=(u == 0),
                                stop=(u == KC2 - 1),
                            )
                            l2_names.add(mm.ins.name)
                        if b2_nz:
                            nc.vector.tensor_scalar_add(
                                ps[:, :nn], ps[:, :nn], b2_sb[:, e, t:t + 1]
                            )
                        nc.scalar.copy(oT_sb[:, t, cc:cc + nn], ps[:, :nn])
                emit_blocks(seg_end[e] // P)

            emit_blocks(NBLK)

    if GATHER:
        # Inject cross-core waits into the scheduled stream. Engines execute
        # their queues in block order, so placing a wait immediately before an
        # instruction in the block puts it immediately before it in that
        # engine's queue.
        bwait = nc.gpsimd.wait_ge(
            nc._bir_kernel_barrier_sem, nc.bir_kernel_barrier_sem_inc
        ).ins
        rwait = nc.tensor.wait_ge(rsem, 2 * (NCORES - 1)).ins
        moved = 0
        for blk in nc.m.functions[0].blocks:
            names = [i.name for i in blk.instructions]
            if trig_name in names:
                cur = next(b for b in nc.m.functions[0].blocks
                           if any(i.name == bwait.name for i in b.instructions))
                cur.instructions.remove(bwait)
                blk.instructions.insert(names.index(trig_name), bwait)
                moved += 1
            names = [i.name for i in blk.instructions]
            idxs = [k for k, i in enumerate(names) if i in l2_names]
            if idxs:
                cur = next(b for b in nc.m.functions[0].blocks
                           if any(i.name == rwait.name for i in b.instructions))
                cur.instructions.remove(rwait)
                blk.instructions.insert(min(idxs), rwait)
                moved += 1
        assert moved == 2, f"wait injection found {moved} anchor sites"

    nc.compile()
    return nc


def kernel(x, W1, b1, W2, b2):
    x = np.asarray(x, dtype=np.float32)
    W1 = np.asarray(W1, dtype=np.float32)
    b1 = np.asarray(b1, dtype=np.float32)
    W2 = np.asarray(W2, dtype=np.float32)
    b2 = np.asarray(b2, dtype=np.float32)

    route = _route(x)
    pad_idx, segs = _plan(route)

    xs = np.ascontiguousarray(x[pad_idx].T).astype(ml_dtypes.bfloat16)
    # (P, KC1, NP) partition-major
    xt = np.ascontiguousarray(
        xs.reshape(KC1, P, NP).transpose(1, 0, 2)
    ).reshape(P, KC1 * NP)

    w1b = W1.astype(ml_dtypes.bfloat16)   # (E, D, H)
    w2b = W2.astype(ml_dtypes.bfloat16)   # (E, H, O)
    ident = np.eye(P, dtype=ml_dtypes.bfloat16)

    b1_nz = bool(np.any(b1))
    b2_nz = bool(np.any(b2))

    nc = _build_program(segs, b1_nz, b2_nz)

    in_maps = []
    for c in range(NCORES):
        # hidden 128-tile held in this core's slot u = pair XOR layout
        if GATHER:
            hb = [2 * (c ^ (u // 2)) + (u % 2) for u in range(KC2)]
        else:
            hb = list(range(KC2))
        loc = hb[:NLOC]
        # W1 slice for the locally computed slots: (E, P, KC1, NLOC*128)
        cols = np.concatenate([np.arange(t * P, (t + 1) * P) for t in loc])
        w1c = w1b[:, :, cols]                      # (E, D, NLOC*P)
        w1c = np.ascontiguousarray(
            w1c.reshape(E, KC1, P, NLOC * P).transpose(0, 2, 1, 3)
        ).reshape(E, P, KC1 * NLOC * P)
        # W2 slice, k-tiles in this core's slot order: (E, P, KC2*OSL)
        sl = slice(c * OSL, (c + 1) * OSL)
        w2c = w2b[:, :, sl].reshape(E, KC2, P, OSL)[:, hb]
        w2c = np.ascontiguousarray(w2c.transpose(0, 2, 1, 3)).reshape(
            E, P, KC2 * OSL
        )
        b1c = b1[:, np.concatenate([np.arange(t * P, (t + 1) * P)
                                    for t in loc])].reshape(E, NLOC, P)
        b1c = np.ascontiguousarray(b1c.transpose(0, 2, 1))
        b2c = np.ascontiguousarray(
            b2[:, sl].reshape(E, NCT, P).transpose(0, 2, 1)
        )
        in_maps.append({
            "xt": xt,
            "w1": w1c,
            "idt": ident,
            "w2": w2c,
            "b1": b1c,
            "b2": b2c,
        })

    trace = os.environ.get("BASSMOE_TRACE", "") == "1"
    if trace:
        trace = _install_ntff_hook()

    res = run_bass_kernel_spmd(
        nc, in_maps, core_ids=list(range(NCORES)), trace=trace,
        tmpdir=os.environ.get("BASSMOE_TRACE_DIR") or None,
    )
    global LAST_EXEC_NS, LAST_TRACE
    LAST_EXEC_NS = res.exec_time_ns
    LAST_TRACE = (res.instructions_and_trace[1]
                  if res.instructions_and_trace else None)

    out_sorted = np.concatenate(
        [res.results[c]["out"].astype(np.float32) for c in range(NCORES)],
        axis=1,
    )
    out = np.empty((B, O), dtype=np.float32)
    out[pad_idx] = out_sorted
    return out.reshape(B, 32, 256)
